# revision 21
# baseline (speedup 1.0000x reference)
"""BERT self-attention (BS=4, SEQ=2048, HID=768, NH=12) on 8 NeuronCores.

Sharding: core c -> batch b = c//2, head-group g = c%2 (6 heads each).

v3: software-pipelined single-phase design.
  - Attention runs in (pair j, q-chunk of 512) tiles.  Per k-block:
    scores for both heads land in one [128,1024] PSUM tile (row-tiled,
    concurrent on the PE), one ACT exp instruction covers both heads
    (the ACT engine is the kernel's throughput floor), and the ctx
    matmuls accumulate [65,512] per head where row 64 is the softmax
    denominator (V carries an appended mask column).
  - QKV projections for pair j+1 are emitted interleaved into the
    attention(j) instruction stream, filling the PE idle slots under
    the ACT-bound steady state and keeping the PE busy enough that the
    HAM clock gate stays at full rate.
  - Drain per (j,qc): reciprocal_approx_fast on the two denominator
    rows, PE broadcast to 64 rows, DVE multiply.  Nothing in the drain
    touches the score-tile ring, so the pipeline never stalls.

PSUM budget (8 banks): proj 2 + scores 4 + ctxA/bcast 1 + ctxB 1.
Biases fold in via an appended ones-row on X^T (contraction 769).
Host does input transposes (free), sharding, and the final
[d,q]->[q,d] untranspose + concat.
"""

from collections import deque

import numpy as np

import concourse.bass as bass
import concourse.tile as tile
from concourse import bacc
from concourse import mybir
from concourse.bass_utils import run_bass_kernel_spmd

F32 = mybir.dt.float32
F16 = mybir.dt.float16
DT_MM = F16
DT_NP = np.float16

BS, SEQ, HID, NH, HD = 4, 2048, 768, 12, 64
NCORES = 8
HPC = 6          # heads per core
FCH = 6          # 128-row chunks of the 768 contraction dim
DSH = HPC * HD   # 384 output features per core


def _body(tc, xt_d, wq_d, wk_d, wv_d, mt_d, ot_d):
    nc = tc.nc
    Exp = mybir.ActivationFunctionType.Exp

    with tc.tile_pool(name="persist", bufs=1) as persist, \
         tc.tile_pool(name="pjp", bufs=2, space="PSUM") as pjp, \
         tc.tile_pool(name="sp", bufs=2, space="PSUM") as sp, \
         tc.tile_pool(name="cpA", bufs=1, space="PSUM") as cpA, \
         tc.tile_pool(name="cpB", bufs=1, space="PSUM") as cpB, \
         tc.tile_pool(name="pp", bufs=2) as pp, \
         tc.tile_pool(name="ctp", bufs=2) as ctp, \
         tc.tile_pool(name="rdp", bufs=2) as rdp, \
         tc.tile_pool(name="osp", bufs=3) as osp:
        # Warm the exp table set ASAP (overlaps the input DMAs).
        dummy = persist.tile([1, 1], F32, tag="dummy")
        nc.vector.memset(dummy, 0.0)
        nc.scalar.activation(out=dummy, in_=dummy, func=Exp)

        mtile = persist.tile([128, 16], DT_MM, tag="mtile")
        nc.sync.dma_start(out=mtile, in_=mt_d[:, :])
        mtf = persist.tile([128, 16], F32, tag="mtf")
        nc.vector.tensor_copy(out=mtf, in_=mtile)

        qt = [persist.tile([128, SEQ], DT_MM, tag=f"qt{j}", name=f"qt{j}")
              for j in range(3)]
        kt = [persist.tile([128, SEQ], DT_MM, tag=f"kt{j}", name=f"kt{j}")
              for j in range(3)]
        # V: [k, pair, head-half, 65] = per pair [h0 d 0:64 | mask | h1 d | mask]
        vt = persist.tile([128, 16, 3, 2, 65], DT_MM, tag="vt")
        xt1 = persist.tile([1, SEQ], DT_MM, tag="x6")
        nc.sync.dma_start(out=xt1, in_=xt_d[768:769, :])

        # interleave W and X DMAs so the first projection chunk can start
        # as soon as the first (w, x) tile pair lands
        xts = []
        wmap = {"q": [], "k": [], "v": []}
        wdram = {"q": wq_d, "k": wk_d, "v": wv_d}
        for f in range(FCH):
            t = persist.tile([128, DSH], DT_MM, tag=f"wq{f}", name=f"wq{f}")
            nc.sync.dma_start(out=t, in_=wq_d[f * 128:(f + 1) * 128, :])
            wmap["q"].append(t)
            t = persist.tile([128, SEQ], DT_MM, tag=f"x{f}", name=f"x{f}")
            nc.sync.dma_start(out=t, in_=xt_d[f * 128:(f + 1) * 128, :])
            xts.append(t)
        b = persist.tile([1, DSH], DT_MM, tag="wqb", name="wqb")
        nc.sync.dma_start(out=b, in_=wq_d[768:769, :])
        wmap["q"].append(b)
        for nm in ("k", "v"):
            for f in range(FCH):
                t = persist.tile([128, DSH], DT_MM, tag=f"w{nm}{f}",
                                 name=f"w{nm}{f}")
                nc.sync.dma_start(out=t, in_=wdram[nm][f * 128:(f + 1) * 128, :])
                wmap[nm].append(t)
            b = persist.tile([1, DSH], DT_MM, tag=f"w{nm}b", name=f"w{nm}b")
            nc.sync.dma_start(out=b, in_=wdram[nm][768:769, :])
            wmap[nm].append(b)

        # mask columns of V (written once: all 6 head-halves)
        for j in range(3):
            for hh in range(2):
                nc.vector.tensor_copy(out=vt[:, :, j, hh, 64], in_=mtf)

        # ---- projection chunk emitters (PSUM via the 2-bank pjp ring) ----
        def v_chunk(kb):
            # all 3 pairs at once: [128 k, 384 d] per k-block
            ks = slice(kb * 128, (kb + 1) * 128)
            wt = wmap["v"]
            ps = pjp.tile([128, 3, 2, 64], F32, tag="pj", name="pj")
            for f in range(FCH):
                nc.tensor.matmul(ps, lhsT=xts[f][:, ks],
                                 rhs=wt[f],
                                 start=(f == 0), stop=False)
            nc.tensor.matmul(ps, lhsT=xt1[:, ks],
                             rhs=wt[6], start=False, stop=True)
            for j in range(3):
                # strided write skips the mask column at free offset 64
                nc.vector.tensor_scalar_mul(
                    out=vt[:, kb, j, :, 0:64],
                    in0=ps[:, j, :, :],
                    scalar1=mtf[:, kb:kb + 1])

        def qk_chunk(nm, j, qc):
            # qc indexes 256-wide q-chunks (0..7): small pops keep the
            # exp pipeline's PE-FIFO injections short
            js = slice(j * 128, (j + 1) * 128)
            qs = slice(qc * 256, (qc + 1) * 256)
            wt = wmap[nm]
            ps = pjp.tile([128, 512], F32, tag="pj", name="pj")
            for f in range(FCH):
                nc.tensor.matmul(ps[:, 0:256], lhsT=wt[f][:, js],
                                 rhs=xts[f][:, qs],
                                 start=(f == 0), stop=False)
            nc.tensor.matmul(ps[:, 0:256], lhsT=wt[6][:, js], rhs=xt1[:, qs],
                             start=False, stop=True)
            dst = qt[j] if nm == "q" else kt[j]
            nc.vector.tensor_copy(out=dst[:, qs], in_=ps[:, 0:256])

        def proj_chunks(j):
            out = []
            for nm in ("q", "k"):
                for qc in range(8):
                    out.append(lambda nm=nm, qc=qc: qk_chunk(nm, j, qc))
            return deque(out)

        # lead-in: pair-0 Q for qc0 plus ALL of pair-0 K (scores at any qc
        # read the full key sequence); remaining Q chunks interleave
        qk_chunk("q", 0, 0)
        qk_chunk("q", 0, 1)
        for qc in range(8):
            qk_chunk("k", 0, qc)
        pending = deque()
        for qc in range(2, 8):
            pending.append(lambda qc=qc: qk_chunk("q", 0, qc))

        # drain finisher (bc broadcast + multiplies), delayed into the next
        # q-chunk's kb loop so the PE FIFO never stalls on the DVE chain
        finisher = [None]

        def run_finisher():
            if finisher[0] is not None:
                finisher[0]()
                finisher[0] = None

        for j in range(3):
            h0, h1 = 2 * j, 2 * j + 1
            if j < 2:
                pending.extend(proj_chunks(j + 1))
            ostage = {h: osp.tile([64, SEQ], F32, tag="os", name=f"os{h}")
                      for h in (h0, h1)}
            for qc in range(4):
                qs = slice(qc * 512, (qc + 1) * 512)
                ctxA = cpA.tile([65, 512], F32, tag="cA", name="ctxA")
                ctxB = cpB.tile([65, 512], F32, tag="cB", name="ctxB")
                pabs = [None, None]   # pab of kb-1, kb
                first_j0 = (j == 0 and qc == 0)
                for kb in range(16):
                    ks = slice(kb * 128, (kb + 1) * 128)
                    if first_j0:
                        # V for all pairs, just in time for ctx(kb)
                        v_chunk(kb)
                    sab = sp.tile([128, 1024], F32, tag="s", name="sab")
                    nc.tensor.matmul(sab[:, 0:512],
                                     lhsT=kt[j][0:64, ks],
                                     rhs=qt[j][0:64, qs],
                                     start=True, stop=True)
                    nc.tensor.matmul(sab[:, 512:1024],
                                     lhsT=kt[j][64:128, ks],
                                     rhs=qt[j][64:128, qs],
                                     start=True, stop=True)
                    pab = pp.tile([128, 1024], DT_MM, tag="p", name="pab")
                    nc.scalar.activation(out=pab, in_=sab, func=Exp,
                                         scale=0.125)
                    pabs[1] = pab
                    if kb == 2:
                        run_finisher()
                    # ctx one step behind scores: the PE FIFO never waits
                    # on the exp that was just queued
                    if kb > 0:
                        st, sp_ = (kb == 1), False
                        pprev = pabs[0]
                        nc.tensor.matmul(ctxA, lhsT=vt[:, kb - 1, j, 0, :],
                                         rhs=pprev[:, 0:512],
                                         start=st, stop=sp_,
                                         skip_group_check=True)
                        nc.tensor.matmul(ctxB, lhsT=vt[:, kb - 1, j, 1, :],
                                         rhs=pprev[:, 512:1024],
                                         start=st, stop=sp_,
                                         skip_group_check=True)
                    pabs[0] = pab
                    if pending and (kb % 4 == 3 if first_j0 else kb % 2 == 1):
                        pending.popleft()()
                nc.tensor.matmul(ctxA, lhsT=vt[:, 15, j, 0, :],
                                 rhs=pabs[0][:, 0:512],
                                 start=False, stop=True,
                                 skip_group_check=True)
                nc.tensor.matmul(ctxB, lhsT=vt[:, 15, j, 1, :],
                                 rhs=pabs[0][:, 512:1024],
                                 start=False, stop=True,
                                 skip_group_check=True)
                # ---- drain part 1: DVE only ----
                ct = ctp.tile([128, 512], F32, tag="ct", name="ct")
                nc.vector.tensor_copy(out=ct[0:64, :], in_=ctxA[0:64, :])
                nc.vector.tensor_copy(out=ct[64:128, :], in_=ctxB[0:64, :])
                rd = rdp.tile([33, 512], F32, tag="rd", name="rd")
                nc.vector.tensor_copy(out=rd[0:1, :], in_=ctxA[64:65, :])
                nc.vector.tensor_copy(out=rd[32:33, :], in_=ctxB[64:65, :])
                rcp = rdp.tile([33, 512], F32, tag="rcp", name="rcp")
                nc.vector.reciprocal_approx_fast(out=rcp, in_=rd)
                rcpA = rdp.tile([1, 512], DT_MM, tag="rcpA", name="rcpA")
                nc.vector.tensor_copy(out=rcpA, in_=rcp[0:1, :])
                rcpB = rdp.tile([1, 512], DT_MM, tag="rcpB", name="rcpB")
                nc.vector.tensor_copy(out=rcpB, in_=rcp[32:33, :])

                def fin(ct=ct, rcpA=rcpA, rcpB=rcpB, qs=qs,
                        osA=ostage[h0], osB=ostage[h1]):
                    bc = pjp.tile([128, 512], F32, tag="pj", name="bc")
                    nc.tensor.matmul(bc[0:64, :], lhsT=xt1[:, 0:64],
                                     rhs=rcpA, start=True, stop=True,
                                     tile_position=(0, 0),
                                     skip_group_check=True)
                    nc.tensor.matmul(bc[64:128, :], lhsT=xt1[:, 0:64],
                                     rhs=rcpB, start=True, stop=True,
                                     tile_position=(0, 64),
                                     skip_group_check=True)
                    nc.vector.tensor_mul(out=osA[:, qs],
                                         in0=ct[0:64, :], in1=bc[0:64, :])
                    nc.vector.tensor_mul(out=osB[:, qs],
                                         in0=ct[64:128, :], in1=bc[64:128, :])
                finisher[0] = fin
            while pending:
                pending.popleft()()
            # qc3's finisher must precede the ostage DMA emission (the DMA
            # only orders against prior writers in program order)
            run_finisher()
            for h in (h0, h1):
                nc.sync.dma_start(out=ot_d[h], in_=ostage[h])


def build_nc():
    nc = bacc.Bacc("TRN2")
    xt_d = nc.declare_dram_parameter("xt", [HID + 1, SEQ], DT_MM, isOutput=False)
    wq_d = nc.declare_dram_parameter("wqT", [HID + 1, DSH], DT_MM, isOutput=False)
    wk_d = nc.declare_dram_parameter("wkT", [HID + 1, DSH], DT_MM, isOutput=False)
    wv_d = nc.declare_dram_parameter("wvT", [HID + 1, DSH], DT_MM, isOutput=False)
    mt_d = nc.declare_dram_parameter("mt", [128, 16], DT_MM, isOutput=False)
    ot_d = nc.declare_dram_parameter("OT", [HPC, HD, SEQ], F32, isOutput=True)
    with tile.TileContext(nc) as tc:
        _body(tc, xt_d, wq_d, wk_d, wv_d, mt_d, ot_d)
    nc.finalize()
    return nc


_NC_CACHE = None


def _get_nc():
    global _NC_CACHE
    if _NC_CACHE is None:
        _NC_CACHE = build_nc()
    return _NC_CACHE


def make_in_maps(hidden_states, attention_mask, Wq, bq, Wk, bk, Wv, bv):
    in_maps = []
    for c in range(NCORES):
        b, g = c // 2, c % 2
        hs = slice(g * DSH, (g + 1) * DSH)
        xt = np.empty((HID + 1, SEQ), DT_NP)
        xt[:HID] = hidden_states[b].T
        xt[HID] = 1.0
        m = (attention_mask[b, 0, 0] > -1).astype(DT_NP)
        mt = np.ascontiguousarray(m.reshape(16, 128).T)

        def aug(W, bias):
            wa = np.empty((HID + 1, DSH), DT_NP)
            wa[:HID] = W[hs, :].T
            wa[HID] = bias[hs]
            return wa

        in_maps.append({
            "xt": np.ascontiguousarray(xt),
            "wqT": aug(Wq, bq),
            "wkT": aug(Wk, bk),
            "wvT": aug(Wv, bv),
            "mt": mt,
        })
    return in_maps


def gather_out(results):
    out = np.empty((BS, SEQ, HID), np.float32)
    for c in range(NCORES):
        b, g = c // 2, c % 2
        ot = results[c]["OT"]  # [6, 64, 2048]
        out[b, :, g * DSH:(g + 1) * DSH] = (
            ot.transpose(2, 0, 1).reshape(SEQ, DSH)
        )
    return out


def kernel(hidden_states, attention_mask, Wq, bq, Wk, bk, Wv, bv):
    nc = _get_nc()
    in_maps = make_in_maps(hidden_states, attention_mask,
                           Wq, bq, Wk, bk, Wv, bv)
    res = run_bass_kernel_spmd(nc, in_maps, core_ids=list(range(NCORES)))
    return gather_out(res.results)


# revision 25
# speedup vs baseline: 1.1322x; 1.1322x over previous
"""BERT self-attention (BS=4, SEQ=2048, HID=768, NH=12) on 8 NeuronCores.

Sharding: core c -> batch b = c//2, head-group g = c%2 (6 heads each).

v3: software-pipelined single-phase design.
  - Attention runs in (pair j, q-chunk of 512) tiles.  Per k-block:
    scores for both heads land in one [128,1024] PSUM tile (row-tiled,
    concurrent on the PE), one ACT exp instruction covers both heads
    (the ACT engine is the kernel's throughput floor), and the ctx
    matmuls accumulate [65,512] per head where row 64 is the softmax
    denominator (V carries an appended mask column).
  - QKV projections for pair j+1 are emitted interleaved into the
    attention(j) instruction stream, filling the PE idle slots under
    the ACT-bound steady state and keeping the PE busy enough that the
    HAM clock gate stays at full rate.
  - Drain per (j,qc): reciprocal_approx_fast on the two denominator
    rows, PE broadcast to 64 rows, DVE multiply.  Nothing in the drain
    touches the score-tile ring, so the pipeline never stalls.

PSUM budget (8 banks): proj 2 + scores 4 + ctxA/bcast 1 + ctxB 1.
Biases fold in via an appended ones-row on X^T (contraction 769).
Host does input transposes (free), sharding, and the final
[d,q]->[q,d] untranspose + concat.
"""

from collections import deque

import numpy as np

import concourse.bass as bass
import concourse.tile as tile
from concourse import bacc
from concourse import mybir
from concourse.bass_utils import run_bass_kernel_spmd

F32 = mybir.dt.float32
F16 = mybir.dt.float16
DT_MM = F16
DT_NP = np.float16

BS, SEQ, HID, NH, HD = 4, 2048, 768, 12, 64
NCORES = 8
HPC = 6          # heads per core
FCH = 6          # 128-row chunks of the 768 contraction dim
DSH = HPC * HD   # 384 output features per core


def _body(tc, xt_d, wq_d, wk_d, wv_d, mt_d, ot_d):
    nc = tc.nc
    Exp = mybir.ActivationFunctionType.Exp

    with tc.tile_pool(name="persist", bufs=1) as persist, \
         tc.tile_pool(name="pjp", bufs=2, space="PSUM") as pjp, \
         tc.tile_pool(name="sp", bufs=2, space="PSUM") as sp, \
         tc.tile_pool(name="cpA", bufs=1, space="PSUM") as cpA, \
         tc.tile_pool(name="cpB", bufs=1, space="PSUM") as cpB, \
         tc.tile_pool(name="pp", bufs=3) as pp, \
         tc.tile_pool(name="ctp", bufs=2) as ctp, \
         tc.tile_pool(name="rdp", bufs=2) as rdp, \
         tc.tile_pool(name="osp", bufs=3) as osp:
        # Warm the exp table set ASAP (overlaps the input DMAs).
        dummy = persist.tile([1, 1], F32, tag="dummy")
        nc.vector.memset(dummy, 0.0)
        nc.scalar.activation(out=dummy, in_=dummy, func=Exp)

        mtile = persist.tile([128, 16], DT_MM, tag="mtile")
        nc.sync.dma_start(out=mtile, in_=mt_d[:, :])
        mtf = persist.tile([128, 16], F32, tag="mtf")
        nc.vector.tensor_copy(out=mtf, in_=mtile)

        qt = [persist.tile([128, SEQ], DT_MM, tag=f"qt{j}", name=f"qt{j}")
              for j in range(3)]
        kt = [persist.tile([128, SEQ], DT_MM, tag=f"kt{j}", name=f"kt{j}")
              for j in range(3)]
        # V: [k, pair, head-half, 65] = per pair [h0 d 0:64 | mask | h1 d | mask]
        vt = persist.tile([128, 16, 3, 2, 65], DT_MM, tag="vt")
        xt1 = persist.tile([1, SEQ], DT_MM, tag="x6")
        nc.sync.dma_start(out=xt1, in_=xt_d[768:769, :])

        # warmup spin: keep the PE busy while the input DMAs land so the
        # HAM clock gate reaches 2.4 GHz before the real work starts
        for _ in range(28):
            wsp = pjp.tile([64, 512], F32, tag="pj", name="wsp")
            nc.tensor.matmul(wsp, lhsT=xt1[:, 0:64], rhs=xt1[:, 0:512],
                             start=True, stop=True)

        # interleave W and X DMAs so the first projection chunk can start
        # as soon as the first (w, x) tile pair lands
        xts = []
        wmap = {"q": [], "k": [], "v": []}
        wdram = {"q": wq_d, "k": wk_d, "v": wv_d}
        for f in range(FCH):
            t = persist.tile([128, DSH], DT_MM, tag=f"wq{f}", name=f"wq{f}")
            nc.sync.dma_start(out=t, in_=wq_d[f * 128:(f + 1) * 128, :])
            wmap["q"].append(t)
            t = persist.tile([128, SEQ], DT_MM, tag=f"x{f}", name=f"x{f}")
            nc.sync.dma_start(out=t, in_=xt_d[f * 128:(f + 1) * 128, :])
            xts.append(t)
        b = persist.tile([1, DSH], DT_MM, tag="wqb", name="wqb")
        nc.sync.dma_start(out=b, in_=wq_d[768:769, :])
        wmap["q"].append(b)
        for nm in ("k", "v"):
            for f in range(FCH):
                t = persist.tile([128, DSH], DT_MM, tag=f"w{nm}{f}",
                                 name=f"w{nm}{f}")
                nc.sync.dma_start(out=t, in_=wdram[nm][f * 128:(f + 1) * 128, :])
                wmap[nm].append(t)
            b = persist.tile([1, DSH], DT_MM, tag=f"w{nm}b", name=f"w{nm}b")
            nc.sync.dma_start(out=b, in_=wdram[nm][768:769, :])
            wmap[nm].append(b)

        # mask columns of V (written once: all 6 head-halves)
        for j in range(3):
            for hh in range(2):
                nc.vector.tensor_copy(out=vt[:, :, j, hh, 64], in_=mtf)

        # ---- projection chunk emitters (PSUM via the 2-bank pjp ring) ----
        def v_chunk(kb):
            # all 3 pairs at once: [128 k, 384 d] per k-block
            ks = slice(kb * 128, (kb + 1) * 128)
            wt = wmap["v"]
            ps = pjp.tile([128, 3, 2, 64], F32, tag="pj", name="pj")
            for f in range(FCH):
                nc.tensor.matmul(ps, lhsT=xts[f][:, ks],
                                 rhs=wt[f],
                                 start=(f == 0), stop=False)
            nc.tensor.matmul(ps, lhsT=xt1[:, ks],
                             rhs=wt[6], start=False, stop=True)
            for j in range(3):
                # strided write skips the mask column at free offset 64
                nc.vector.tensor_scalar_mul(
                    out=vt[:, kb, j, :, 0:64],
                    in0=ps[:, j, :, :],
                    scalar1=mtf[:, kb:kb + 1])

        def qk_chunk(nm, j, qc):
            # qc indexes 256-wide q-chunks (0..7): small pops keep the
            # exp pipeline's PE-FIFO injections short
            js = slice(j * 128, (j + 1) * 128)
            qs = slice(qc * 256, (qc + 1) * 256)
            wt = wmap[nm]
            ps = pjp.tile([128, 512], F32, tag="pj", name="pj")
            for f in range(FCH):
                nc.tensor.matmul(ps[:, 0:256], lhsT=wt[f][:, js],
                                 rhs=xts[f][:, qs],
                                 start=(f == 0), stop=False)
            nc.tensor.matmul(ps[:, 0:256], lhsT=wt[6][:, js], rhs=xt1[:, qs],
                             start=False, stop=True)
            dst = qt[j] if nm == "q" else kt[j]
            nc.vector.tensor_copy(out=dst[:, qs], in_=ps[:, 0:256])

        def proj_chunks(j):
            out = []
            for nm in ("q", "k"):
                for qc in range(8):
                    out.append(lambda nm=nm, qc=qc: qk_chunk(nm, j, qc))
            return deque(out)

        # lead-in: pair-0 Q for qc0 plus ALL of pair-0 K (scores at any qc
        # read the full key sequence); remaining Q chunks interleave
        qk_chunk("q", 0, 0)
        qk_chunk("q", 0, 1)
        for qc in range(8):
            qk_chunk("k", 0, qc)
        pending = deque()
        for qc in range(2, 8):
            pending.append(lambda qc=qc: qk_chunk("q", 0, qc))

        # drain finisher (bc broadcast + multiplies), delayed into the next
        # q-chunk's kb loop so the PE FIFO never stalls on the DVE chain
        finisher = [None]

        def run_finisher():
            if finisher[0] is not None:
                finisher[0]()
                finisher[0] = None

        for j in range(3):
            h0, h1 = 2 * j, 2 * j + 1
            if j < 2:
                pending.extend(proj_chunks(j + 1))
            ostage = {h: osp.tile([64, SEQ], F32, tag="os", name=f"os{h}")
                      for h in (h0, h1)}
            for qc in range(4):
                qs = slice(qc * 512, (qc + 1) * 512)
                ctxA = cpA.tile([65, 512], F32, tag="cA", name="ctxA")
                ctxB = cpB.tile([65, 512], F32, tag="cB", name="ctxB")
                first_j0 = (j == 0 and qc == 0)

                def scores(kb):
                    ks = slice(kb * 128, (kb + 1) * 128)
                    sab = sp.tile([128, 1024], F32, tag="s", name="sab")
                    nc.tensor.matmul(sab[:, 0:512],
                                     lhsT=kt[j][0:64, ks],
                                     rhs=qt[j][0:64, qs],
                                     start=True, stop=True)
                    nc.tensor.matmul(sab[:, 512:1024],
                                     lhsT=kt[j][64:128, ks],
                                     rhs=qt[j][64:128, qs],
                                     start=True, stop=True)
                    return sab

                # scores run two k-blocks ahead of exp so neither engine
                # ever waits on the other's just-queued work
                sring = {0: scores(0), 1: scores(1)}
                pab_prev = None
                for kb in range(16):
                    if first_j0:
                        # V for all pairs, just in time for ctx(kb)
                        v_chunk(kb)
                    pab = pp.tile([128, 1024], DT_MM, tag="p", name="pab")
                    nc.scalar.activation(out=pab, in_=sring.pop(kb),
                                         func=Exp, scale=0.125)
                    if kb < 14:
                        sring[kb + 2] = scores(kb + 2)
                    if kb == 2:
                        run_finisher()
                    if kb > 0:
                        st = (kb == 1)
                        nc.tensor.matmul(ctxA, lhsT=vt[:, kb - 1, j, 0, :],
                                         rhs=pab_prev[:, 0:512],
                                         start=st, stop=False,
                                         skip_group_check=True)
                        nc.tensor.matmul(ctxB, lhsT=vt[:, kb - 1, j, 1, :],
                                         rhs=pab_prev[:, 512:1024],
                                         start=st, stop=False,
                                         skip_group_check=True)
                    pab_prev = pab
                    if pending and (kb % 4 == 3 if first_j0 else kb % 2 == 1):
                        pending.popleft()()
                nc.tensor.matmul(ctxA, lhsT=vt[:, 15, j, 0, :],
                                 rhs=pab_prev[:, 0:512],
                                 start=False, stop=True,
                                 skip_group_check=True)
                nc.tensor.matmul(ctxB, lhsT=vt[:, 15, j, 1, :],
                                 rhs=pab_prev[:, 512:1024],
                                 start=False, stop=True,
                                 skip_group_check=True)
                # ---- drain part 1: DVE only ----
                ct = ctp.tile([128, 512], F32, tag="ct", name="ct")
                nc.vector.tensor_copy(out=ct[0:64, :], in_=ctxA[0:64, :])
                nc.vector.tensor_copy(out=ct[64:128, :], in_=ctxB[0:64, :])
                rd = rdp.tile([33, 512], F32, tag="rd", name="rd")
                nc.vector.tensor_copy(out=rd[0:1, :], in_=ctxA[64:65, :])
                nc.vector.tensor_copy(out=rd[32:33, :], in_=ctxB[64:65, :])
                rcp = rdp.tile([33, 512], F32, tag="rcp", name="rcp")
                nc.vector.reciprocal_approx_fast(out=rcp, in_=rd)
                rcpA = rdp.tile([1, 512], DT_MM, tag="rcpA", name="rcpA")
                nc.vector.tensor_copy(out=rcpA, in_=rcp[0:1, :])
                rcpB = rdp.tile([1, 512], DT_MM, tag="rcpB", name="rcpB")
                nc.vector.tensor_copy(out=rcpB, in_=rcp[32:33, :])

                def fin(ct=ct, rcpA=rcpA, rcpB=rcpB, qs=qs,
                        osA=ostage[h0], osB=ostage[h1], h0=h0, h1=h1):
                    bc = pjp.tile([128, 512], F32, tag="pj", name="bc")
                    nc.tensor.matmul(bc[0:64, :], lhsT=xt1[:, 0:64],
                                     rhs=rcpA, start=True, stop=True,
                                     tile_position=(0, 0),
                                     skip_group_check=True)
                    nc.tensor.matmul(bc[64:128, :], lhsT=xt1[:, 0:64],
                                     rhs=rcpB, start=True, stop=True,
                                     tile_position=(0, 64),
                                     skip_group_check=True)
                    nc.vector.tensor_mul(out=osA[:, qs],
                                         in0=ct[0:64, :], in1=bc[0:64, :])
                    nc.vector.tensor_mul(out=osB[:, qs],
                                         in0=ct[64:128, :], in1=bc[64:128, :])
                    nc.sync.dma_start(out=ot_d[h0][:, qs], in_=osA[:, qs])
                    nc.sync.dma_start(out=ot_d[h1][:, qs], in_=osB[:, qs])
                finisher[0] = fin
            while pending:
                pending.popleft()()
            if j == 2:
                run_finisher()


def build_nc():
    nc = bacc.Bacc("TRN2")
    xt_d = nc.declare_dram_parameter("xt", [HID + 1, SEQ], DT_MM, isOutput=False)
    wq_d = nc.declare_dram_parameter("wqT", [HID + 1, DSH], DT_MM, isOutput=False)
    wk_d = nc.declare_dram_parameter("wkT", [HID + 1, DSH], DT_MM, isOutput=False)
    wv_d = nc.declare_dram_parameter("wvT", [HID + 1, DSH], DT_MM, isOutput=False)
    mt_d = nc.declare_dram_parameter("mt", [128, 16], DT_MM, isOutput=False)
    ot_d = nc.declare_dram_parameter("OT", [HPC, HD, SEQ], F32, isOutput=True)
    with tile.TileContext(nc) as tc:
        _body(tc, xt_d, wq_d, wk_d, wv_d, mt_d, ot_d)
    nc.finalize()
    return nc


_NC_CACHE = None


def _get_nc():
    global _NC_CACHE
    if _NC_CACHE is None:
        _NC_CACHE = build_nc()
    return _NC_CACHE


def make_in_maps(hidden_states, attention_mask, Wq, bq, Wk, bk, Wv, bv):
    in_maps = []
    for c in range(NCORES):
        b, g = c // 2, c % 2
        hs = slice(g * DSH, (g + 1) * DSH)
        xt = np.empty((HID + 1, SEQ), DT_NP)
        xt[:HID] = hidden_states[b].T
        xt[HID] = 1.0
        m = (attention_mask[b, 0, 0] > -1).astype(DT_NP)
        mt = np.ascontiguousarray(m.reshape(16, 128).T)

        def aug(W, bias):
            wa = np.empty((HID + 1, DSH), DT_NP)
            wa[:HID] = W[hs, :].T
            wa[HID] = bias[hs]
            return wa

        in_maps.append({
            "xt": np.ascontiguousarray(xt),
            "wqT": aug(Wq, bq),
            "wkT": aug(Wk, bk),
            "wvT": aug(Wv, bv),
            "mt": mt,
        })
    return in_maps


def gather_out(results):
    out = np.empty((BS, SEQ, HID), np.float32)
    for c in range(NCORES):
        b, g = c // 2, c % 2
        ot = results[c]["OT"]  # [6, 64, 2048]
        out[b, :, g * DSH:(g + 1) * DSH] = (
            ot.transpose(2, 0, 1).reshape(SEQ, DSH)
        )
    return out


def kernel(hidden_states, attention_mask, Wq, bq, Wk, bk, Wv, bv):
    nc = _get_nc()
    in_maps = make_in_maps(hidden_states, attention_mask,
                           Wq, bq, Wk, bk, Wv, bv)
    res = run_bass_kernel_spmd(nc, in_maps, core_ids=list(range(NCORES)))
    return gather_out(res.results)
